# revision 28
# baseline (speedup 1.0000x reference)
"""MoE feed-forward (top-2 routing, E=8 experts) on 8 trn2 NeuronCores.

Strategy: expert parallelism (1 expert per core), QUARTER-PIPELINED combine.
  - Router is token-sharded; x arrives host-pretransposed so logitsT [E, TPC]
    needs only 16 fp32 matmuls + 8 tiny PE transposes (no 128x128 transposes).
  - Tokens are split into 4 QUARTERS (per-core token columns 2q, 2q+1).
    Each quarter has its own compaction table (CAPQ=640 slots/expert,
    4*640 = 2560 = the reference capacity, so total FFN work is unchanged)
    and its own combine buffer [2048, D].
  - Per-(expert, quarter) counts are AllGather'd (tiny) ASAP after the
    router; local ranks are computed while the AllGather flies.
    Keep/drop uses the FULL global rank < 2560 (exact reference semantics);
    the quarter slot index uses the within-quarter global rank < CAPQ.
  - 16 indirect scatters (8 cols x 2 experts) write (row+1, gate) records
    into the 4 quarter tables; ReduceScatter(add) over the expert axis hands
    each core its expert's [CAPQ, 2] slots per quarter.  RS-t0 is triggered
    after only the first 4 scatters so quarter 0's FFN starts early.
  - Expert FFN in bf16 (fp32 PSUM): 5 uniform groups of 512 slots spanning
    quarter boundaries (keeps weight-load count minimal).  Output rows are
    weighted and scattered into the owning quarter's combine buffer; as soon
    as a quarter's last slot tile is scattered, its combine ReduceScatter
    (4MB bf16) is triggered -- overlapping under the remaining FFN groups.
    Only the LAST quarter's RS (~35us) stays on the critical path.
  - Combine: RS(add) sums the two expert contributions per token; each core
    converts its 256-row quarter shard to f32 into out_shard.  Copyouts are
    emitted last and fenced on the final group's output so the scheduler
    cannot park their RS-waiting loads ahead of FFN gelu work.

Token layout: core c, local col j in [0,8), partition p -> t = 1024c+128j+p.
Quarter q = j//2; row within quarter buffer = 256c + 128(j%2) + p.
"""
import numpy as np
import ml_dtypes

import concourse.tile as tile
from concourse import bass, bacc, mybir
from concourse.bass_utils import run_bass_kernel_spmd
from concourse.masks import make_identity, make_upper_triangular

N_CORES = 8
P = 128
E = 8
K = 2
D = 1024
F = 2048
B, S = 4, 2048
T = B * S
TPC = T // N_CORES         # 1024 tokens per core
CAP = 2560                 # reference per-expert capacity
NQ = 4                     # quarters
CAPQ = CAP // NQ           # 640 slots per expert per quarter
QT = TPC // NQ             # 256 tokens per core per quarter
RQ = N_CORES * QT          # 2048 rows per quarter buffer
NSQ = CAPQ // P            # 5 slot tiles per quarter
GRP = 512                  # moving free dim per matmul group
NGRP = CAP // GRP          # 5 groups
DC = D // P                # 8 d-chunks
FC = F // P                # 16 f-chunks
NL = TPC // P              # 8 local token columns
W = E * NL                 # 64
PAD_TOK = 65536            # padding marker (> RQ-1 -> OOB, DMA skipped)
BIG = 1.0e6                # drop clamp -> lands OOB of table
f32 = mybir.dt.float32
bf16 = mybir.dt.bfloat16
i32 = mybir.dt.int32


def build_kernel():
    nc = bacc.Bacc(num_devices=N_CORES)

    # ---------------- parameters ----------------
    xT_p = nc.declare_dram_parameter("xT_p", [P, DC * TPC], f32, isOutput=False)
    xqs = [nc.declare_dram_parameter(f"xq{q}", [RQ, D], bf16, isOutput=False)
           for q in range(NQ)]
    rw = nc.declare_dram_parameter("rw", [D, E], f32, isOutput=False)
    rb_c = nc.declare_dram_parameter("rb_c", [E, 1], f32, isOutput=False)
    my_e = nc.declare_dram_parameter("my_e", [P, 1], f32, isOutput=False)
    myrow = nc.declare_dram_parameter("myrow", [E, 1], f32, isOutput=False)
    w1_p = nc.declare_dram_parameter("w1_p", [P, DC * FC * P], bf16, isOutput=False)
    b1_p = nc.declare_dram_parameter("b1_p", [P, FC], f32, isOutput=False)
    w2_p = nc.declare_dram_parameter("w2_p", [P, FC * DC * P], bf16, isOutput=False)
    b2_p = nc.declare_dram_parameter("b2_p", [P, DC], f32, isOutput=False)
    out_shard = nc.declare_dram_parameter("out_shard", [TPC, D], f32, isOutput=True)

    # ---------------- internal DRAM ----------------
    cnt_in = nc.dram_tensor("cnt_in", [E * NQ], f32)
    cnt_all = nc.dram_tensor("cnt_all", [N_CORES, E * NQ], f32, addr_space="Shared")
    pts = [nc.dram_tensor(f"pt{q}", [E * CAPQ, 2], f32) for q in range(NQ)]
    sms = [nc.dram_tensor(f"sm{q}", [CAPQ, 2], f32) for q in range(NQ)]
    rss = [nc.dram_tensor(f"rs{q}", [RQ, D], bf16) for q in range(NQ)]
    rsos = [nc.dram_tensor(f"rso{q}", [QT, D], bf16) for q in range(NQ)]

    with tile.TileContext(nc) as tc:
        with tc.tile_pool(name="const", bufs=1) as cpool:
            ident = cpool.tile([P, P], f32)
            make_identity(nc, ident[:])
            tri = cpool.tile([P, P], f32)
            make_upper_triangular(nc, tri[:], val=1.0, diag=False)  # tri[p,i]=1 iff p<i
            ones_col = cpool.tile([P, 1], f32)
            nc.gpsimd.memset(ones_col[:], 1.0)
            ones_row1 = cpool.tile([1, P], f32)
            nc.gpsimd.memset(ones_row1[:], 1.0)
            rb_sb = cpool.tile([E, 1], f32)
            nc.sync.dma_start(out=rb_sb[:], in_=rb_c.ap())
            mye_sb = cpool.tile([P, 1], f32)
            nc.sync.dma_start(out=mye_sb[:], in_=my_e.ap())
            myrow_sb = cpool.tile([E, 1], f32)
            nc.sync.dma_start(out=myrow_sb[:], in_=myrow.ap())
            rw_sb = cpool.tile([P, DC, E], f32)
            nc.sync.dma_start(out=rw_sb[:], in_=rw.ap().rearrange("(c p) e -> p c e", p=P))
            warm = cpool.tile([1, 2], f32)
            nc.scalar.activation(out=warm[:, 0:1], in_=ones_col[0:1, 0:1],
                                 func=mybir.ActivationFunctionType.Sigmoid)
            nc.scalar.activation(out=warm[:, 1:2], in_=ones_col[0:1, 0:1],
                                 func=mybir.ActivationFunctionType.Gelu)
            tokf = cpool.tile([P, 2], f32)
            toki = cpool.tile([P, 2], i32)
            nc.gpsimd.iota(toki[:], pattern=[[P, 2]], base=0, channel_multiplier=1)
            nc.vector.tensor_copy(tokf[:], toki[:])

            # ---------- router: logitsT = rw^T @ xT  (fp32, exact) ----------
            meta_sb = cpool.tile([P, 4 * E], f32)
            mxs = cpool.tile([P, NL, 8], f32)
            mis = cpool.tile([P, NL, 8], mybir.dt.uint32)
            zseed = cpool.tile([P, 1], f32)
            with tc.tile_pool(name="rt", bufs=1) as rt, \
                 tc.tile_pool(name="rtp", bufs=1, space="PSUM") as rtp:
                # split the xT load in halves so matmuls start ~10us earlier
                xTh = [rt.tile([P, DC, TPC // 2], f32, name=f"xTh{h}")
                       for h in range(2)]
                HW2 = DC * (TPC // 2)
                for h in range(2):
                    nc.sync.dma_start(
                        out=xTh[h][:],
                        in_=xT_p.ap()[:, h * HW2:(h + 1) * HW2]
                        .rearrange("p (c t) -> p c t", c=DC))
                # heavy-load gate: ready as soon as the router inputs are in
                nc.vector.tensor_scalar(out=zseed[:], in0=xTh[0][:, 0, 0:1],
                                        scalar1=0.0, scalar2=None,
                                        op0=mybir.AluOpType.mult)
                lgS = rt.tile([E, TPC], f32)
                for h in range(2):
                    lgp = rtp.tile([E, TPC // 2], f32, space="PSUM",
                                   tag=f"lg{h}", bufs=1)
                    for dci in range(DC):
                        nc.tensor.matmul(
                            out=lgp[:], lhsT=rw_sb[:, dci, :],
                            rhs=xTh[h][:, dci, :],
                            start=(dci == 0), stop=(dci == DC - 1))
                    nc.vector.tensor_scalar(
                        out=lgS[:, h * (TPC // 2):(h + 1) * (TPC // 2)],
                        in0=lgp[:], scalar1=rb_sb[:, 0:1], scalar2=None,
                        op0=mybir.AluOpType.add)
                lsb = rt.tile([P, NL, E], f32)
                for g in range(NL):
                    tp = rtp.tile([P, E], f32, space="PSUM", tag="tp", bufs=2)
                    nc.tensor.transpose(out=tp[:], in_=lgS[:, g * P:(g + 1) * P],
                                        identity=ident[0:E, 0:E])
                    nc.vector.tensor_copy(lsb[:, g, :], tp[:])
                for g in range(NL):
                    nc.vector.max_with_indices(mxs[:, g, :], mis[:, g, :],
                                               lsb[:, g, :])
                # fields: E1 | E2 | G1 | G2 at cols 0:8, 8:16, 16:24, 24:32
                nc.vector.tensor_copy(meta_sb[:, 0:E], mis[:, :, 0])
                nc.vector.tensor_copy(meta_sb[:, E:2 * E], mis[:, :, 1])
                diffs = rt.tile([P, E], f32)
                nc.vector.tensor_tensor(out=diffs[:], in0=mxs[:, :, 0],
                                        in1=mxs[:, :, 1],
                                        op=mybir.AluOpType.subtract)
                nc.scalar.activation(out=meta_sb[:, 2 * E:3 * E], in_=diffs[:],
                                     func=mybir.ActivationFunctionType.Sigmoid)
                nc.vector.tensor_scalar(out=meta_sb[:, 3 * E:4 * E],
                                        in0=meta_sb[:, 2 * E:3 * E],
                                        scalar1=-1.0, scalar2=1.0,
                                        op0=mybir.AluOpType.mult,
                                        op1=mybir.AluOpType.add)

            # scatter source + per-quarter staging tiles (outlive the mt pool)
            oA = cpool.tile([P, NL], i32)
            oB = cpool.tile([P, NL], i32)
            payA = cpool.tile([P, 2 * NL], f32)
            payB = cpool.tile([P, 2 * NL], f32)
            stg_o = [cpool.tile([P, 4], i32, name=f"stgo{q}") for q in range(NQ)]
            stg_p = [cpool.tile([P, 8], f32, name=f"stgp{q}") for q in range(NQ)]

            def scatter_quarter(q, seed):
                # stage this quarter's offsets/payloads on gpsimd; the seed
                # (previous quarter's gathered rows) keeps quarters 1-3 out of
                # the DGE ring until quarter q-1's critical path has issued
                if seed is not None:
                    nc.vector.tensor_copy(stg_o[q][0:1, 0:1], seed[0:1, 0:1])
                nc.gpsimd.tensor_copy(stg_o[q][:, 0:2], oA[:, 2 * q:2 * q + 2])
                nc.gpsimd.tensor_copy(stg_o[q][:, 2:4], oB[:, 2 * q:2 * q + 2])
                nc.gpsimd.tensor_copy(stg_p[q][:, 0:4], payA[:, 4 * q:4 * q + 4])
                nc.gpsimd.tensor_copy(stg_p[q][:, 4:8], payB[:, 4 * q:4 * q + 4])
                for i in range(2):
                    for c0, p0 in ((0, 0), (2, 4)):
                        nc.gpsimd.indirect_dma_start(
                            out=pts[q].ap(),
                            out_offset=bass.IndirectOffsetOnAxis(
                                ap=stg_o[q][:, c0 + i:c0 + i + 1], axis=0),
                            in_=stg_p[q][:, p0 + 2 * i:p0 + 2 * i + 2],
                            in_offset=None,
                            bounds_check=E * CAPQ - 1,
                            oob_is_err=False,
                        )
                nc.gpsimd.collective_compute(
                    "ReduceScatter", mybir.AluOpType.add,
                    replica_groups=[list(range(N_CORES))],
                    ins=[pts[q].ap().opt()], outs=[sms[q].ap().opt()],
                )

            # ---------- masks + per-(e,q) counts -> AllGather ASAP ----------
            with tc.tile_pool(name="mt", bufs=1) as mt, \
                 tc.tile_pool(name="mtp", bufs=1, space="PSUM") as mtp:
                E1b = mt.tile([P, W], f32)
                E2b = mt.tile([P, W], f32)
                eidx = mt.tile([P, W], f32)
                nc.vector.tensor_copy(
                    E1b[:].rearrange("p (e c) -> p e c", e=E),
                    meta_sb[:, 0:E].rearrange("p (e c) -> p e c", e=1)
                    .to_broadcast([P, E, NL]))
                nc.vector.tensor_copy(
                    E2b[:].rearrange("p (e c) -> p e c", e=E),
                    meta_sb[:, E:2 * E].rearrange("p (e c) -> p e c", e=1)
                    .to_broadcast([P, E, NL]))
                for e in range(E):
                    nc.vector.memset(eidx[:, e * NL:(e + 1) * NL], float(e))
                m1b = mt.tile([P, W], f32)
                m2b = mt.tile([P, W], f32)
                maskb = mt.tile([P, W], f32)
                nc.vector.tensor_tensor(out=m1b[:], in0=E1b[:], in1=eidx[:],
                                        op=mybir.AluOpType.is_equal)
                nc.vector.tensor_tensor(out=m2b[:], in0=E2b[:], in1=eidx[:],
                                        op=mybir.AluOpType.is_equal)
                nc.vector.tensor_tensor(out=maskb[:], in0=m1b[:], in1=m2b[:],
                                        op=mybir.AluOpType.add)
                # per-column counts, then per-(e, q) counts -> AllGather NOW
                cps = mtp.tile([1, W], f32, space="PSUM", tag="cps")
                nc.tensor.matmul(out=cps[:], lhsT=ones_col[:], rhs=maskb[:],
                                 start=True, stop=True)
                ctot = mt.tile([1, W], f32)
                nc.vector.tensor_copy(ctot[:], cps[:])
                cnt32 = mt.tile([1, E * NQ], f32)
                nc.vector.tensor_tensor(out=cnt32[:], in0=ctot[:, 0:W:2],
                                        in1=ctot[:, 1:W:2],
                                        op=mybir.AluOpType.add)
                cnt32g = mt.tile([1, E * NQ], f32)
                nc.gpsimd.tensor_copy(cnt32g[:], cnt32[:])
                nc.scalar.dma_start(out=cnt_in.ap(), in_=cnt32g[:])
                nc.gpsimd.collective_compute(
                    "AllGather", mybir.AluOpType.bypass,
                    replica_groups=[list(range(N_CORES))],
                    ins=[cnt_in.ap().opt()], outs=[cnt_all.ap().opt()],
                )

                # ---- local ranks while the AllGather flies ----
                rpsF = mtp.tile([P, W], f32, space="PSUM", tag="rpsF")
                nc.tensor.matmul(out=rpsF[:], lhsT=tri[:], rhs=maskb[:],
                                 start=True, stop=False)
                rpsQ = mtp.tile([P, W], f32, space="PSUM", tag="rpsQ")
                nc.tensor.matmul(out=rpsQ[:], lhsT=tri[:], rhs=maskb[:],
                                 start=True, stop=False)
                cinc = mt.tile([1, W], f32)
                nc.vector.tensor_tensor_scan(out=cinc[:], data0=ctot[:], data1=ctot[:],
                                             initial=0.0, op0=mybir.AluOpType.add,
                                             op1=mybir.AluOpType.bypass)
                bases = mt.tile([1, E], f32)
                nc.vector.tensor_copy(bases[:, 1:E], cinc[0:1, NL - 1:W - NL:NL])
                nc.vector.memset(bases[:, 0:1], 0.0)
                cexc = mt.tile([1, W], f32)
                nc.vector.tensor_tensor(out=cexc[:], in0=cinc[:], in1=ctot[:],
                                        op=mybir.AluOpType.subtract)
                # quarter-start exclusive counts (per expert block, col pairs)
                qstart = mt.tile([1, W], f32)
                nc.vector.tensor_copy(qstart[:, 0:W:2], cexc[:, 0:W:2])
                nc.vector.tensor_copy(qstart[:, 1:W:2], cexc[:, 0:W:2])

                # zero the 4 combine buffers (16MB, flat contiguous APs so the
                # descriptor count stays tiny) + w1/w2 loads; gated on the xT
                # input load via zseed so they stay out of the router's window
                ZR = 2                              # rows per partition per call
                ztile = cpool.tile([P, ZR * D], bf16)
                nc.vector.memset(ztile[:], 0.0)
                nc.vector.tensor_copy(ztile[:, 0:1], zseed[:])
                for q in range(NQ):
                    for zi in range(RQ // (ZR * P)):
                        nc.sync.dma_start(
                            out=rss[q].ap()[zi * ZR * P:(zi + 1) * ZR * P, :]
                            .rearrange("(p a) d -> p (a d)", p=P),
                            in_=ztile[:])
                # zero the 4 quarter tables (gpsimd, tiny)
                zpt = mt.tile([P, (E * CAPQ // P) * 2], f32)
                nc.gpsimd.memset(zpt[:], 0.0)
                for q in range(NQ):
                    nc.gpsimd.dma_start(
                        out=pts[q].ap().rearrange("(p a) w -> p (a w)", p=P),
                        in_=zpt[:])

                w1_sb = cpool.tile([P, DC, FC, P], bf16)
                nc.vector.tensor_copy(w1_sb[:, 0, 0, 0:1], zseed[:])
                nc.sync.dma_start(out=w1_sb[:], in_=w1_p.ap().rearrange(
                    "p (dc fc q) -> p dc fc q", dc=DC, fc=FC))
                w2_sb = cpool.tile([P, FC, DC, P], bf16)
                nc.vector.tensor_copy(w2_sb[:, 0, 0, 0:1], zseed[:])
                nc.sync.dma_start(out=w2_sb[:], in_=w2_p.ap().rearrange(
                    "p (fc dc q) -> p fc dc q", fc=FC, dc=DC))
                b1_sb = cpool.tile([P, FC], f32)
                nc.sync.dma_start(out=b1_sb[:], in_=b1_p.ap())
                b2_sb = cpool.tile([P, DC], f32)
                nc.sync.dma_start(out=b2_sb[:], in_=b2_p.ap())

                # ---- post-AllGather: global bases ----
                # scalar queue: its zero-writes drain by ~60us, so this load
                # is not stuck behind the w1/w2 streams on the sync queue
                cnt8 = mt.tile([N_CORES, E * NQ], f32)
                nc.scalar.dma_start(out=cnt8[:], in_=cnt_all.ap())
                pref_ps = mtp.tile([N_CORES, E * NQ], f32, space="PSUM", tag="pref")
                nc.tensor.matmul(out=pref_ps[:], lhsT=tri[0:N_CORES, 0:N_CORES],
                                 rhs=cnt8[:], start=True, stop=True)
                pref_sb = mt.tile([N_CORES, E * NQ], f32)
                nc.vector.tensor_copy(pref_sb[:], pref_ps[:])
                mybase_ps = mtp.tile([1, E * NQ], f32, space="PSUM", tag="mybase")
                nc.tensor.matmul(out=mybase_ps[:], lhsT=myrow_sb[:], rhs=pref_sb[:],
                                 start=True, stop=True)
                mq32 = mt.tile([1, E * NQ], f32)      # (e, q) e-major
                nc.vector.tensor_copy(mq32[:], mybase_ps[:])
                # full-rank base per expert = sum over quarters
                myF = mt.tile([1, E], f32)
                nc.vector.tensor_tensor(out=myF[:], in0=mq32[:, 0:E * NQ:NQ],
                                        in1=mq32[:, 1:E * NQ:NQ],
                                        op=mybir.AluOpType.add)
                tmpF = mt.tile([1, E], f32)
                nc.vector.tensor_tensor(out=tmpF[:], in0=mq32[:, 2:E * NQ:NQ],
                                        in1=mq32[:, 3:E * NQ:NQ],
                                        op=mybir.AluOpType.add)
                nc.vector.tensor_tensor(out=myF[:], in0=myF[:], in1=tmpF[:],
                                        op=mybir.AluOpType.add)
                # adjF[e*8+j] = cexc - bases[e] + myF[e]
                adjF = mt.tile([1, W], f32)
                nc.vector.tensor_tensor(
                    out=adjF[:].rearrange("a (e c) -> a e c", e=E),
                    in0=myF[:].rearrange("a (e c) -> a e c", c=1)
                    .to_broadcast([1, E, NL]),
                    in1=bases[:].rearrange("a (e c) -> a e c", c=1)
                    .to_broadcast([1, E, NL]),
                    op=mybir.AluOpType.subtract)
                nc.vector.tensor_tensor(out=adjF[:], in0=adjF[:], in1=cexc[:],
                                        op=mybir.AluOpType.add)
                # adjQ[e*8+j] = cexc - qstart + mq32[e, j//2]
                adjQ = mt.tile([1, W], f32)
                nc.vector.tensor_copy(
                    adjQ[:].rearrange("a (e q u) -> a e q u", e=E, q=NQ),
                    mq32[:].rearrange("a (e q u) -> a e q u", e=E, u=1)
                    .to_broadcast([1, E, NQ, 2]))
                nc.vector.tensor_tensor(out=adjQ[:], in0=adjQ[:], in1=cexc[:],
                                        op=mybir.AluOpType.add)
                nc.vector.tensor_tensor(out=adjQ[:], in0=adjQ[:], in1=qstart[:],
                                        op=mybir.AluOpType.subtract)
                # broadcast-add across partitions via PE, finish both ranks
                nc.tensor.matmul(out=rpsF[:], lhsT=ones_row1[:], rhs=adjF[:],
                                 start=False, stop=True)
                nc.tensor.matmul(out=rpsQ[:], lhsT=ones_row1[:], rhs=adjQ[:],
                                 start=False, stop=True)
                rkF = mt.tile([P, W], f32)
                nc.vector.tensor_copy(rkF[:], rpsF[:])
                rkQ = mt.tile([P, W], f32)
                nc.vector.tensor_copy(rkQ[:], rpsQ[:])
                # offsets: e*CAPQ + slotQ, +BIG if dropped (full rank >= CAP,
                # exact reference semantics) or slot overflow (slotQ >= CAPQ)
                drop = mt.tile([P, W], f32)
                nc.vector.tensor_scalar(out=drop[:], in0=rkF[:], scalar1=float(CAP),
                                        scalar2=BIG, op0=mybir.AluOpType.is_ge,
                                        op1=mybir.AluOpType.mult)
                nc.vector.tensor_tensor(out=rkQ[:], in0=rkQ[:], in1=drop[:],
                                        op=mybir.AluOpType.add)
                nc.vector.tensor_scalar(out=drop[:], in0=rkQ[:], scalar1=float(CAPQ),
                                        scalar2=BIG, op0=mybir.AluOpType.is_ge,
                                        op1=mybir.AluOpType.mult)
                nc.vector.tensor_tensor(out=rkQ[:], in0=rkQ[:], in1=drop[:],
                                        op=mybir.AluOpType.add)
                nc.vector.tensor_scalar(out=drop[:], in0=eidx[:], scalar1=float(CAPQ),
                                        scalar2=None, op0=mybir.AluOpType.mult)
                nc.vector.tensor_tensor(out=rkQ[:], in0=rkQ[:], in1=drop[:],
                                        op=mybir.AluOpType.add)
                # one-hot select across expert blocks
                pA = mt.tile([P, W], f32)
                pB = mt.tile([P, W], f32)
                nc.vector.tensor_tensor(out=pA[:], in0=rkQ[:], in1=m1b[:],
                                        op=mybir.AluOpType.mult)
                nc.vector.tensor_tensor(out=pB[:], in0=rkQ[:], in1=m2b[:],
                                        op=mybir.AluOpType.mult)
                for src in (pA, pB):
                    for e in range(1, E):
                        nc.vector.tensor_tensor(
                            out=src[:, 0:NL], in0=src[:, 0:NL],
                            in1=src[:, e * NL:(e + 1) * NL],
                            op=mybir.AluOpType.add)
                # payload rows: row-in-quarter + 1 = 256c + 128(j%2) + p + 1
                mye256 = mt.tile([P, 1], f32)
                nc.vector.tensor_scalar(out=mye256[:], in0=mye_sb[:],
                                        scalar1=float(QT), scalar2=1.0,
                                        op0=mybir.AluOpType.mult,
                                        op1=mybir.AluOpType.add)
                tokp1 = mt.tile([P, NL], f32)
                nc.vector.tensor_copy(
                    tokp1[:].rearrange("p (q i) -> p q i", q=NQ),
                    tokf[:].rearrange("p (q i) -> p q i", q=1)
                    .to_broadcast([P, NQ, 2]))
                nc.vector.tensor_scalar(out=tokp1[:], in0=tokp1[:],
                                        scalar1=mye256[:, 0:1], scalar2=None,
                                        op0=mybir.AluOpType.add)
                nc.vector.tensor_copy(oA[:], pA[:, 0:NL])
                nc.vector.tensor_copy(oB[:], pB[:, 0:NL])
                nc.vector.tensor_copy(payA[:, 0:2 * NL:2], tokp1[:])
                nc.vector.tensor_copy(payA[:, 1:2 * NL:2], meta_sb[:, 2 * E:3 * E])
                nc.vector.tensor_copy(payB[:, 0:2 * NL:2], tokp1[:])
                nc.vector.tensor_copy(payB[:, 1:2 * NL:2], meta_sb[:, 3 * E:4 * E])
                # quarter 0's staging + 4 scatters + RS-t0 ONLY: with no other
                # DGE calls in flight, the trigger waits on exactly these 4
                # completions (no semaphore-ring aliasing) and fires ~30us
                # earlier; quarters 1-3 are chained into the FFN section.
                scatter_quarter(0, None)

            # ---------- per-quarter decode tiles ----------
            slot_toks = [cpool.tile([P, NSQ], i32, name=f"stok{q}")
                         for q in range(NQ)]
            slot_ws = [cpool.tile([P, NSQ], f32, name=f"sw{q}")
                       for q in range(NQ)]

            # =========== expert FFN (bf16, fp32 accumulate) ===========
            with tc.tile_pool(name="dq", bufs=1) as dq, \
                 tc.tile_pool(name="ffn", bufs=2) as ffn, \
                 tc.tile_pool(name="ffg", bufs=2) as ffg, \
                 tc.tile_pool(name="ffp", bufs=4, space="PSUM") as ffp:
                xgTs = [ffn.tile([P, DC, GRP], bf16, name=f"xgT_{g}", bufs=1)
                        for g in range(NGRP)]

                def gather_quarter(q):
                    # decode my slot table for quarter q (smf load on scalar:
                    # it waits on RS-t_q and must not block the gather queue)
                    smf = dq.tile([P, NSQ, 2], f32, tag="smf", bufs=2)
                    nc.scalar.dma_start(out=smf[:], in_=sms[q].ap().rearrange(
                        "(s p) w -> p s w", p=P))
                    nc.vector.tensor_copy(slot_ws[q][:], smf[:, :, 1])
                    tdec = dq.tile([P, NSQ], f32, tag="tdec", bufs=2)
                    empt = dq.tile([P, NSQ], f32, tag="empt", bufs=2)
                    nc.vector.tensor_scalar(out=empt[:], in0=smf[:, :, 0],
                                            scalar1=0.0,
                                            scalar2=float(PAD_TOK + 1),
                                            op0=mybir.AluOpType.is_equal,
                                            op1=mybir.AluOpType.mult)
                    nc.vector.tensor_scalar(out=tdec[:], in0=smf[:, :, 0],
                                            scalar1=-1.0, scalar2=None,
                                            op0=mybir.AluOpType.add)
                    nc.vector.tensor_tensor(out=tdec[:], in0=tdec[:], in1=empt[:],
                                            op=mybir.AluOpType.add)
                    nc.vector.tensor_copy(slot_toks[q][:], tdec[:])
                    for s in range(NSQ):
                        gt = NSQ * q + s
                        xg = ffg.tile([P, D], bf16, tag="xg", bufs=4)
                        nc.gpsimd.indirect_dma_start(
                            out=xg[:], out_offset=None,
                            in_=xqs[q].ap(),
                            in_offset=bass.IndirectOffsetOnAxis(
                                ap=slot_toks[q][:, s:s + 1], axis=0),
                            bounds_check=RQ - 1,
                            oob_is_err=False,
                        )
                        # xbar: xgT[p, dc, st*128+t] = xg[t, dc*128+p]
                        nc.sync.dma_start_transpose(
                            out=xgTs[gt // 4][:, :, (gt % 4) * P:(gt % 4 + 1) * P],
                            in_=xg[:])
                    return xg

                seed = gather_quarter(0)
                for q in range(1, NQ):
                    scatter_quarter(q, seed)
                    seed = gather_quarter(q)

                for g in range(NGRP):
                    xgT = xgTs[g]
                    # mm1 + gelu -> hT
                    hT = ffn.tile([P, FC, GRP], bf16, tag="hT")
                    for fci in range(FC):
                        hp = ffp.tile([P, GRP], f32, space="PSUM", tag="hp")
                        for dci in range(DC):
                            nc.tensor.matmul(out=hp[:],
                                             lhsT=w1_sb[:, dci, fci, :],
                                             rhs=xgT[:, dci, :],
                                             start=(dci == 0), stop=(dci == DC - 1))
                        nc.scalar.activation(out=hT[:, fci, :], in_=hp[:],
                                             func=mybir.ActivationFunctionType.Gelu,
                                             bias=b1_sb[:, fci:fci + 1], scale=1.0)
                    # mm2 -> (+bias)*gate -> oT; xbar-transpose immediately
                    oT = ffn.tile([P, DC, GRP], bf16, tag="oT")
                    owg = ffn.tile([P, GRP // P, D], bf16, tag="owg")
                    for dci in range(DC):
                        op = ffp.tile([P, GRP], f32, space="PSUM", tag="op")
                        for fci in range(FC):
                            nc.tensor.matmul(out=op[:],
                                             lhsT=w2_sb[:, fci, dci, :],
                                             rhs=hT[:, fci, :],
                                             start=(fci == 0), stop=(fci == FC - 1))
                        nc.vector.tensor_scalar(out=oT[:, dci, :], in0=op[:],
                                                scalar1=b2_sb[:, dci:dci + 1],
                                                scalar2=None,
                                                op0=mybir.AluOpType.add)
                        # owg[p, st, dc*128+jj] = oT[jj, dc, st*128+p]
                        nc.sync.dma_start_transpose(
                            out=owg[:, :, dci * P:(dci + 1) * P],
                            in_=oT[:, dci, :])
                    for st in range(GRP // P):
                        gt = g * (GRP // P) + st
                        q, s = gt // NSQ, gt % NSQ
                        ow = ffg.tile([P, D], bf16, tag="ow", bufs=3)
                        nc.vector.tensor_scalar(out=ow[:], in0=owg[:, st, :],
                                                scalar1=slot_ws[q][:, s:s + 1],
                                                scalar2=None,
                                                op0=mybir.AluOpType.mult)
                        nc.gpsimd.indirect_dma_start(
                            out=rss[q].ap(),
                            out_offset=bass.IndirectOffsetOnAxis(
                                ap=slot_toks[q][:, s:s + 1], axis=0),
                            in_=ow[:],
                            in_offset=None,
                            bounds_check=RQ - 1,
                            oob_is_err=False,
                        )
                        if s == NSQ - 1:
                            # quarter q fully scattered -> combine RS
                            nc.gpsimd.collective_compute(
                                "ReduceScatter", mybir.AluOpType.add,
                                replica_groups=[list(range(N_CORES))],
                                ins=[rss[q].ap().opt()], outs=[rsos[q].ap().opt()],
                            )
                # copyouts LAST, fenced on a gpsimd sentinel emitted after
                # the final combine-RS trigger: their DMA traffic then runs
                # under RS-rs3's window instead of interleaving with (and
                # slowing) the last group's combine scatters
                sent = ffg.tile([1, 1], f32, tag="sent", bufs=1)
                nc.gpsimd.tensor_copy(sent[:], ones_col[0:1, 0:1])
                fence = ffg.tile([1, 1], f32, tag="fence", bufs=1)
                nc.vector.tensor_copy(fence[:], sent[:])
                for q in range(NQ):
                    for h in range(QT // P):
                        eng = nc.scalar if h % 2 == 0 else nc.sync
                        cb = ffg.tile([P, D], bf16, tag="cb", bufs=2)
                        nc.vector.tensor_copy(cb[0:1, 0:1], fence[:])
                        eng.dma_start(
                            out=cb[:], in_=rsos[q].ap()[h * P:(h + 1) * P, :])
                        ob = ffg.tile([P, D], f32, tag="ob", bufs=2)
                        nc.vector.tensor_copy(ob[:], cb[:])
                        eng.dma_start(
                            out=out_shard.ap()
                            [q * QT + h * P:q * QT + (h + 1) * P, :],
                            in_=ob[:])

    nc.finalize()
    return nc


_NC_CACHE = None
TRACE = False
LAST_EXEC_NS = None
LAST_TRACE_DIR = None


def kernel(x, router_w, router_b, w1, b1, w2, b2):
    global _NC_CACHE
    x = np.ascontiguousarray(np.asarray(x, np.float32))
    router_w = np.ascontiguousarray(np.asarray(router_w, np.float32))
    router_b = np.asarray(router_b, np.float32)
    w1 = np.asarray(w1, np.float32)
    b1 = np.asarray(b1, np.float32)
    w2 = np.asarray(w2, np.float32)
    b2 = np.asarray(b2, np.float32)

    xf = x.reshape(T, D)
    xbf = xf.astype(ml_dtypes.bfloat16)
    # quarter-permuted replicas: xq[q][c*QT + j] = x[c*TPC + q*QT + j]
    xq4 = xbf.reshape(N_CORES, NQ, QT, D)
    xq_arrs = [np.ascontiguousarray(xq4[:, q].reshape(RQ, D)) for q in range(NQ)]

    in_maps = []
    for c in range(N_CORES):
        xs = xf[c * TPC:(c + 1) * TPC]                      # [TPC, D]
        # two contiguous halves: [P, (h c t)] with t in [0, TPC/2)
        xT = np.ascontiguousarray(
            xs.T.reshape(DC, P, 2, TPC // 2).transpose(1, 2, 0, 3)
        ).reshape(P, DC * TPC)
        w1t = np.ascontiguousarray(
            w1[c].reshape(DC, P, FC, P).transpose(1, 0, 2, 3)
        ).astype(ml_dtypes.bfloat16).reshape(P, DC * FC * P)
        w2t = np.ascontiguousarray(
            w2[c].reshape(FC, P, DC, P).transpose(1, 0, 2, 3)
        ).astype(ml_dtypes.bfloat16).reshape(P, FC * DC * P)
        myr = np.zeros((E, 1), np.float32)
        myr[c, 0] = 1.0
        im = {
            "xT_p": xT,
            "rw": router_w,
            "rb_c": np.ascontiguousarray(router_b.reshape(E, 1)),
            "my_e": np.full((P, 1), float(c), np.float32),
            "myrow": myr,
            "w1_p": w1t,
            "b1_p": np.ascontiguousarray(b1[c].reshape(FC, P).T),
            "w2_p": w2t,
            "b2_p": np.ascontiguousarray(b2[c].reshape(DC, P).T),
        }
        for q in range(NQ):
            im[f"xq{q}"] = xq_arrs[q]
        in_maps.append(im)

    global LAST_EXEC_NS, LAST_TRACE_DIR
    if _NC_CACHE is None:
        _NC_CACHE = build_kernel()
    import tempfile
    td = tempfile.mkdtemp(prefix="moe_trace_") if TRACE else None
    res = run_bass_kernel_spmd(_NC_CACHE, in_maps, list(range(N_CORES)),
                               trace=TRACE, tmpdir=td)
    LAST_EXEC_NS = getattr(res, "exec_time_ns", None)
    LAST_TRACE_DIR = td
    out = np.concatenate([res.results[c]["out_shard"] for c in range(N_CORES)], axis=0)
    return out.reshape(B, S, D)


# revision 29
# speedup vs baseline: 1.1237x; 1.1237x over previous
"""MoE feed-forward (top-2 routing, E=8 experts) on 8 trn2 NeuronCores.

Strategy: expert parallelism (1 expert per core), QUARTER-PIPELINED combine.
  - Router is token-sharded; x arrives host-pretransposed so logitsT [E, TPC]
    needs only 16 fp32 matmuls + 8 tiny PE transposes (no 128x128 transposes).
  - Tokens are split into 4 QUARTERS (per-core token columns 2q, 2q+1).
    Each quarter has its own compaction table (CAPQ=640 slots/expert,
    4*640 = 2560 = the reference capacity, so total FFN work is unchanged)
    and its own combine buffer [2048, D].
  - Per-(expert, quarter) counts are AllGather'd (tiny) ASAP after the
    router; local ranks are computed while the AllGather flies.
    Keep/drop uses the FULL global rank < 2560 (exact reference semantics);
    the quarter slot index uses the within-quarter global rank < CAPQ.
  - 16 indirect scatters (8 cols x 2 experts) write (row+1, gate) records
    into the 4 quarter tables; ReduceScatter(add) over the expert axis hands
    each core its expert's [CAPQ, 2] slots per quarter.  RS-t0 is triggered
    after only the first 4 scatters so quarter 0's FFN starts early.
  - Expert FFN in bf16 (fp32 PSUM): 5 uniform groups of 512 slots spanning
    quarter boundaries (keeps weight-load count minimal).  Output rows are
    weighted and scattered into the owning quarter's combine buffer; as soon
    as a quarter's last slot tile is scattered, its combine ReduceScatter
    (4MB bf16) is triggered -- overlapping under the remaining FFN groups.
    Only the LAST quarter's RS (~35us) stays on the critical path.
  - Combine: RS(add) sums the two expert contributions per token; each core
    converts its 256-row quarter shard to f32 into out_shard.  Copyouts are
    emitted last and fenced on the final group's output so the scheduler
    cannot park their RS-waiting loads ahead of FFN gelu work.

Token layout: core c, local col j in [0,8), partition p -> t = 1024c+128j+p.
Quarter q = j//2; row within quarter buffer = 256c + 128(j%2) + p.
"""
import numpy as np
import ml_dtypes

import concourse.tile as tile
from concourse import bass, bacc, mybir
from concourse.bass_utils import run_bass_kernel_spmd
from concourse.masks import make_identity, make_upper_triangular

N_CORES = 8
P = 128
E = 8
K = 2
D = 1024
F = 2048
B, S = 4, 2048
T = B * S
TPC = T // N_CORES         # 1024 tokens per core
CAP = 2560                 # reference per-expert capacity
NQ = 4                     # quarters
CAPQ = CAP // NQ           # 640 slots per expert per quarter
QT = TPC // NQ             # 256 tokens per core per quarter
RQ = N_CORES * QT          # 2048 rows per quarter buffer
NSQ = CAPQ // P            # 5 slot tiles per quarter
GRP = 512                  # moving free dim per matmul group
NGRP = CAP // GRP          # 5 groups
DC = D // P                # 8 d-chunks
FC = F // P                # 16 f-chunks
NL = TPC // P              # 8 local token columns
W = E * NL                 # 64
PAD_TOK = 65536            # padding marker (> RQ-1 -> OOB, DMA skipped)
BIG = 1.0e6                # drop clamp -> lands OOB of table
f32 = mybir.dt.float32
bf16 = mybir.dt.bfloat16
i32 = mybir.dt.int32


def build_kernel():
    nc = bacc.Bacc(num_devices=N_CORES)

    # ---------------- parameters ----------------
    xT_p = nc.declare_dram_parameter("xT_p", [P, DC * TPC], f32, isOutput=False)
    xqs = [nc.declare_dram_parameter(f"xq{q}", [RQ, D], bf16, isOutput=False)
           for q in range(NQ)]
    rw = nc.declare_dram_parameter("rw", [D, E], f32, isOutput=False)
    rb_c = nc.declare_dram_parameter("rb_c", [E, 1], f32, isOutput=False)
    my_e = nc.declare_dram_parameter("my_e", [P, 1], f32, isOutput=False)
    myrow = nc.declare_dram_parameter("myrow", [E, 1], f32, isOutput=False)
    w1_p = nc.declare_dram_parameter("w1_p", [P, DC * FC * P], bf16, isOutput=False)
    b1_p = nc.declare_dram_parameter("b1_p", [P, FC], f32, isOutput=False)
    w2_p = nc.declare_dram_parameter("w2_p", [P, FC * DC * P], bf16, isOutput=False)
    b2_p = nc.declare_dram_parameter("b2_p", [P, DC], f32, isOutput=False)
    out_shard = nc.declare_dram_parameter("out_shard", [TPC, D], f32, isOutput=True)

    # ---------------- internal DRAM ----------------
    cnt_in = nc.dram_tensor("cnt_in", [E * NQ], f32)
    cnt_all = nc.dram_tensor("cnt_all", [N_CORES, E * NQ], f32, addr_space="Shared")
    pts = [nc.dram_tensor(f"pt{q}", [E * CAPQ, 2], f32) for q in range(NQ)]
    sms = [nc.dram_tensor(f"sm{q}", [CAPQ, 2], f32) for q in range(NQ)]
    rss = [nc.dram_tensor(f"rs{q}", [RQ, D], bf16) for q in range(NQ)]
    rsos = [nc.dram_tensor(f"rso{q}", [QT, D], bf16) for q in range(NQ)]

    with tile.TileContext(nc) as tc:
        with tc.tile_pool(name="const", bufs=1) as cpool:
            ident = cpool.tile([P, P], f32)
            make_identity(nc, ident[:])
            tri = cpool.tile([P, P], f32)
            make_upper_triangular(nc, tri[:], val=1.0, diag=False)  # tri[p,i]=1 iff p<i
            ones_col = cpool.tile([P, 1], f32)
            nc.gpsimd.memset(ones_col[:], 1.0)
            ones_row1 = cpool.tile([1, P], f32)
            nc.gpsimd.memset(ones_row1[:], 1.0)
            rb_sb = cpool.tile([E, 1], f32)
            nc.sync.dma_start(out=rb_sb[:], in_=rb_c.ap())
            mye_sb = cpool.tile([P, 1], f32)
            nc.sync.dma_start(out=mye_sb[:], in_=my_e.ap())
            myrow_sb = cpool.tile([E, 1], f32)
            nc.sync.dma_start(out=myrow_sb[:], in_=myrow.ap())
            rw_sb = cpool.tile([P, DC, E], f32)
            nc.sync.dma_start(out=rw_sb[:], in_=rw.ap().rearrange("(c p) e -> p c e", p=P))
            warm = cpool.tile([1, 2], f32)
            nc.scalar.activation(out=warm[:, 0:1], in_=ones_col[0:1, 0:1],
                                 func=mybir.ActivationFunctionType.Sigmoid)
            nc.scalar.activation(out=warm[:, 1:2], in_=ones_col[0:1, 0:1],
                                 func=mybir.ActivationFunctionType.Gelu)
            tokf = cpool.tile([P, 2], f32)
            toki = cpool.tile([P, 2], i32)
            nc.gpsimd.iota(toki[:], pattern=[[P, 2]], base=0, channel_multiplier=1)
            nc.vector.tensor_copy(tokf[:], toki[:])

            # ---------- router: logitsT = rw^T @ xT  (fp32, exact) ----------
            meta_sb = cpool.tile([P, 4 * E], f32)
            mxs = cpool.tile([P, NL, 8], f32)
            mis = cpool.tile([P, NL, 8], mybir.dt.uint32)
            zseed = cpool.tile([P, 1], f32)
            with tc.tile_pool(name="rt", bufs=1) as rt, \
                 tc.tile_pool(name="rtp", bufs=1, space="PSUM") as rtp:
                # split the xT load in halves so matmuls start ~10us earlier
                xTh = [rt.tile([P, DC, TPC // 2], f32, name=f"xTh{h}")
                       for h in range(2)]
                HW2 = DC * (TPC // 2)
                for h in range(2):
                    nc.sync.dma_start(
                        out=xTh[h][:],
                        in_=xT_p.ap()[:, h * HW2:(h + 1) * HW2]
                        .rearrange("p (c t) -> p c t", c=DC))
                # heavy-load gate: ready as soon as the router inputs are in
                nc.vector.tensor_scalar(out=zseed[:], in0=xTh[0][:, 0, 0:1],
                                        scalar1=0.0, scalar2=None,
                                        op0=mybir.AluOpType.mult)
                lgS = rt.tile([E, TPC], f32)
                for h in range(2):
                    lgp = rtp.tile([E, TPC // 2], f32, space="PSUM",
                                   tag=f"lg{h}", bufs=1)
                    for dci in range(DC):
                        nc.tensor.matmul(
                            out=lgp[:], lhsT=rw_sb[:, dci, :],
                            rhs=xTh[h][:, dci, :],
                            start=(dci == 0), stop=(dci == DC - 1))
                    nc.vector.tensor_scalar(
                        out=lgS[:, h * (TPC // 2):(h + 1) * (TPC // 2)],
                        in0=lgp[:], scalar1=rb_sb[:, 0:1], scalar2=None,
                        op0=mybir.AluOpType.add)
                lsb = rt.tile([P, NL, E], f32)
                for g in range(NL):
                    tp = rtp.tile([P, E], f32, space="PSUM", tag="tp", bufs=2)
                    nc.tensor.transpose(out=tp[:], in_=lgS[:, g * P:(g + 1) * P],
                                        identity=ident[0:E, 0:E])
                    nc.vector.tensor_copy(lsb[:, g, :], tp[:])
                for g in range(NL):
                    nc.vector.max_with_indices(mxs[:, g, :], mis[:, g, :],
                                               lsb[:, g, :])
                # fields: E1 | E2 | G1 | G2 at cols 0:8, 8:16, 16:24, 24:32
                nc.vector.tensor_copy(meta_sb[:, 0:E], mis[:, :, 0])
                nc.vector.tensor_copy(meta_sb[:, E:2 * E], mis[:, :, 1])
                diffs = rt.tile([P, E], f32)
                nc.vector.tensor_tensor(out=diffs[:], in0=mxs[:, :, 0],
                                        in1=mxs[:, :, 1],
                                        op=mybir.AluOpType.subtract)
                nc.scalar.activation(out=meta_sb[:, 2 * E:3 * E], in_=diffs[:],
                                     func=mybir.ActivationFunctionType.Sigmoid)
                nc.vector.tensor_scalar(out=meta_sb[:, 3 * E:4 * E],
                                        in0=meta_sb[:, 2 * E:3 * E],
                                        scalar1=-1.0, scalar2=1.0,
                                        op0=mybir.AluOpType.mult,
                                        op1=mybir.AluOpType.add)

            # scatter source + per-quarter staging tiles (outlive the mt pool)
            oA = cpool.tile([P, NL], i32)
            oB = cpool.tile([P, NL], i32)
            payA = cpool.tile([P, 2 * NL], f32)
            payB = cpool.tile([P, 2 * NL], f32)
            stg_o = [cpool.tile([P, 4], i32, name=f"stgo{q}") for q in range(NQ)]
            stg_p = [cpool.tile([P, 8], f32, name=f"stgp{q}") for q in range(NQ)]

            def scatter_quarter(q, seed):
                # stage this quarter's offsets/payloads on gpsimd; the seed
                # (previous quarter's gathered rows) keeps quarters 1-3 out of
                # the DGE ring until quarter q-1's critical path has issued
                if seed is not None:
                    nc.vector.tensor_copy(stg_o[q][0:1, 0:1], seed[0:1, 0:1])
                nc.gpsimd.tensor_copy(stg_o[q][:, 0:2], oA[:, 2 * q:2 * q + 2])
                nc.gpsimd.tensor_copy(stg_o[q][:, 2:4], oB[:, 2 * q:2 * q + 2])
                nc.gpsimd.tensor_copy(stg_p[q][:, 0:4], payA[:, 4 * q:4 * q + 4])
                nc.gpsimd.tensor_copy(stg_p[q][:, 4:8], payB[:, 4 * q:4 * q + 4])
                for i in range(2):
                    for c0, p0 in ((0, 0), (2, 4)):
                        nc.gpsimd.indirect_dma_start(
                            out=pts[q].ap(),
                            out_offset=bass.IndirectOffsetOnAxis(
                                ap=stg_o[q][:, c0 + i:c0 + i + 1], axis=0),
                            in_=stg_p[q][:, p0 + 2 * i:p0 + 2 * i + 2],
                            in_offset=None,
                            bounds_check=E * CAPQ - 1,
                            oob_is_err=False,
                        )
                nc.gpsimd.collective_compute(
                    "ReduceScatter", mybir.AluOpType.add,
                    replica_groups=[list(range(N_CORES))],
                    ins=[pts[q].ap().opt()], outs=[sms[q].ap().opt()],
                )

            # ---------- masks + per-(e,q) counts -> AllGather ASAP ----------
            with tc.tile_pool(name="mt", bufs=1) as mt, \
                 tc.tile_pool(name="mtp", bufs=1, space="PSUM") as mtp:
                E1b = mt.tile([P, W], f32)
                E2b = mt.tile([P, W], f32)
                eidx = mt.tile([P, W], f32)
                nc.vector.tensor_copy(
                    E1b[:].rearrange("p (e c) -> p e c", e=E),
                    meta_sb[:, 0:E].rearrange("p (e c) -> p e c", e=1)
                    .to_broadcast([P, E, NL]))
                nc.vector.tensor_copy(
                    E2b[:].rearrange("p (e c) -> p e c", e=E),
                    meta_sb[:, E:2 * E].rearrange("p (e c) -> p e c", e=1)
                    .to_broadcast([P, E, NL]))
                for e in range(E):
                    nc.vector.memset(eidx[:, e * NL:(e + 1) * NL], float(e))
                m1b = mt.tile([P, W], f32)
                m2b = mt.tile([P, W], f32)
                maskb = mt.tile([P, W], f32)
                nc.vector.tensor_tensor(out=m1b[:], in0=E1b[:], in1=eidx[:],
                                        op=mybir.AluOpType.is_equal)
                nc.vector.tensor_tensor(out=m2b[:], in0=E2b[:], in1=eidx[:],
                                        op=mybir.AluOpType.is_equal)
                nc.vector.tensor_tensor(out=maskb[:], in0=m1b[:], in1=m2b[:],
                                        op=mybir.AluOpType.add)
                # per-column counts, then per-(e, q) counts -> AllGather NOW
                cps = mtp.tile([1, W], f32, space="PSUM", tag="cps")
                nc.tensor.matmul(out=cps[:], lhsT=ones_col[:], rhs=maskb[:],
                                 start=True, stop=True)
                ctot = mt.tile([1, W], f32)
                nc.vector.tensor_copy(ctot[:], cps[:])
                cnt32 = mt.tile([1, E * NQ], f32)
                nc.vector.tensor_tensor(out=cnt32[:], in0=ctot[:, 0:W:2],
                                        in1=ctot[:, 1:W:2],
                                        op=mybir.AluOpType.add)
                cnt32g = mt.tile([1, E * NQ], f32)
                nc.gpsimd.tensor_copy(cnt32g[:], cnt32[:])
                nc.scalar.dma_start(out=cnt_in.ap(), in_=cnt32g[:])
                nc.gpsimd.collective_compute(
                    "AllGather", mybir.AluOpType.bypass,
                    replica_groups=[list(range(N_CORES))],
                    ins=[cnt_in.ap().opt()], outs=[cnt_all.ap().opt()],
                )

                # ---- local ranks while the AllGather flies ----
                rpsF = mtp.tile([P, W], f32, space="PSUM", tag="rpsF")
                nc.tensor.matmul(out=rpsF[:], lhsT=tri[:], rhs=maskb[:],
                                 start=True, stop=False)
                rpsQ = mtp.tile([P, W], f32, space="PSUM", tag="rpsQ")
                nc.tensor.matmul(out=rpsQ[:], lhsT=tri[:], rhs=maskb[:],
                                 start=True, stop=False)
                cinc = mt.tile([1, W], f32)
                nc.vector.tensor_tensor_scan(out=cinc[:], data0=ctot[:], data1=ctot[:],
                                             initial=0.0, op0=mybir.AluOpType.add,
                                             op1=mybir.AluOpType.bypass)
                bases = mt.tile([1, E], f32)
                nc.vector.tensor_copy(bases[:, 1:E], cinc[0:1, NL - 1:W - NL:NL])
                nc.vector.memset(bases[:, 0:1], 0.0)
                cexc = mt.tile([1, W], f32)
                nc.vector.tensor_tensor(out=cexc[:], in0=cinc[:], in1=ctot[:],
                                        op=mybir.AluOpType.subtract)
                # quarter-start exclusive counts (per expert block, col pairs)
                qstart = mt.tile([1, W], f32)
                nc.vector.tensor_copy(qstart[:, 0:W:2], cexc[:, 0:W:2])
                nc.vector.tensor_copy(qstart[:, 1:W:2], cexc[:, 0:W:2])

                # zero the 4 combine buffers (16MB, flat contiguous APs so the
                # descriptor count stays tiny) + w1/w2 loads; gated on the xT
                # input load via zseed so they stay out of the router's window
                ZR = 2                              # rows per partition per call
                ztile = cpool.tile([P, ZR * D], bf16)
                nc.vector.memset(ztile[:], 0.0)
                nc.vector.tensor_copy(ztile[:, 0:1], zseed[:])
                for q in range(NQ):
                    for zi in range(RQ // (ZR * P)):
                        nc.sync.dma_start(
                            out=rss[q].ap()[zi * ZR * P:(zi + 1) * ZR * P, :]
                            .rearrange("(p a) d -> p (a d)", p=P),
                            in_=ztile[:])
                # zero the 4 quarter tables (gpsimd, tiny)
                zpt = mt.tile([P, (E * CAPQ // P) * 2], f32)
                nc.gpsimd.memset(zpt[:], 0.0)
                for q in range(NQ):
                    nc.gpsimd.dma_start(
                        out=pts[q].ap().rearrange("(p a) w -> p (a w)", p=P),
                        in_=zpt[:])

                w1_sb = cpool.tile([P, DC, FC, P], bf16)
                nc.vector.tensor_copy(w1_sb[:, 0, 0, 0:1], zseed[:])
                nc.sync.dma_start(out=w1_sb[:], in_=w1_p.ap().rearrange(
                    "p (dc fc q) -> p dc fc q", dc=DC, fc=FC))
                w2_sb = cpool.tile([P, FC, DC, P], bf16)
                nc.vector.tensor_copy(w2_sb[:, 0, 0, 0:1], zseed[:])
                nc.sync.dma_start(out=w2_sb[:], in_=w2_p.ap().rearrange(
                    "p (fc dc q) -> p fc dc q", fc=FC, dc=DC))
                b1_sb = cpool.tile([P, FC], f32)
                nc.sync.dma_start(out=b1_sb[:], in_=b1_p.ap())
                b2_sb = cpool.tile([P, DC], f32)
                nc.sync.dma_start(out=b2_sb[:], in_=b2_p.ap())

                # ---- post-AllGather: global bases ----
                # scalar queue: its zero-writes drain by ~60us, so this load
                # is not stuck behind the w1/w2 streams on the sync queue
                cnt8 = mt.tile([N_CORES, E * NQ], f32)
                nc.scalar.dma_start(out=cnt8[:], in_=cnt_all.ap())
                pref_ps = mtp.tile([N_CORES, E * NQ], f32, space="PSUM", tag="pref")
                nc.tensor.matmul(out=pref_ps[:], lhsT=tri[0:N_CORES, 0:N_CORES],
                                 rhs=cnt8[:], start=True, stop=True)
                pref_sb = mt.tile([N_CORES, E * NQ], f32)
                nc.vector.tensor_copy(pref_sb[:], pref_ps[:])
                mybase_ps = mtp.tile([1, E * NQ], f32, space="PSUM", tag="mybase")
                nc.tensor.matmul(out=mybase_ps[:], lhsT=myrow_sb[:], rhs=pref_sb[:],
                                 start=True, stop=True)
                mq32 = mt.tile([1, E * NQ], f32)      # (e, q) e-major
                nc.vector.tensor_copy(mq32[:], mybase_ps[:])
                # full-rank base per expert = sum over quarters
                myF = mt.tile([1, E], f32)
                nc.vector.tensor_tensor(out=myF[:], in0=mq32[:, 0:E * NQ:NQ],
                                        in1=mq32[:, 1:E * NQ:NQ],
                                        op=mybir.AluOpType.add)
                tmpF = mt.tile([1, E], f32)
                nc.vector.tensor_tensor(out=tmpF[:], in0=mq32[:, 2:E * NQ:NQ],
                                        in1=mq32[:, 3:E * NQ:NQ],
                                        op=mybir.AluOpType.add)
                nc.vector.tensor_tensor(out=myF[:], in0=myF[:], in1=tmpF[:],
                                        op=mybir.AluOpType.add)
                # adjF[e*8+j] = cexc - bases[e] + myF[e]
                adjF = mt.tile([1, W], f32)
                nc.vector.tensor_tensor(
                    out=adjF[:].rearrange("a (e c) -> a e c", e=E),
                    in0=myF[:].rearrange("a (e c) -> a e c", c=1)
                    .to_broadcast([1, E, NL]),
                    in1=bases[:].rearrange("a (e c) -> a e c", c=1)
                    .to_broadcast([1, E, NL]),
                    op=mybir.AluOpType.subtract)
                nc.vector.tensor_tensor(out=adjF[:], in0=adjF[:], in1=cexc[:],
                                        op=mybir.AluOpType.add)
                # adjQ[e*8+j] = cexc - qstart + mq32[e, j//2]
                adjQ = mt.tile([1, W], f32)
                nc.vector.tensor_copy(
                    adjQ[:].rearrange("a (e q u) -> a e q u", e=E, q=NQ),
                    mq32[:].rearrange("a (e q u) -> a e q u", e=E, u=1)
                    .to_broadcast([1, E, NQ, 2]))
                nc.vector.tensor_tensor(out=adjQ[:], in0=adjQ[:], in1=cexc[:],
                                        op=mybir.AluOpType.add)
                nc.vector.tensor_tensor(out=adjQ[:], in0=adjQ[:], in1=qstart[:],
                                        op=mybir.AluOpType.subtract)
                # broadcast-add across partitions via PE, finish both ranks
                nc.tensor.matmul(out=rpsF[:], lhsT=ones_row1[:], rhs=adjF[:],
                                 start=False, stop=True)
                nc.tensor.matmul(out=rpsQ[:], lhsT=ones_row1[:], rhs=adjQ[:],
                                 start=False, stop=True)
                rkF = mt.tile([P, W], f32)
                nc.vector.tensor_copy(rkF[:], rpsF[:])
                rkQ = mt.tile([P, W], f32)
                nc.vector.tensor_copy(rkQ[:], rpsQ[:])
                # offsets: e*CAPQ + slotQ, +BIG if dropped (full rank >= CAP,
                # exact reference semantics) or slot overflow (slotQ >= CAPQ)
                drop = mt.tile([P, W], f32)
                nc.vector.tensor_scalar(out=drop[:], in0=rkF[:], scalar1=float(CAP),
                                        scalar2=BIG, op0=mybir.AluOpType.is_ge,
                                        op1=mybir.AluOpType.mult)
                nc.vector.tensor_tensor(out=rkQ[:], in0=rkQ[:], in1=drop[:],
                                        op=mybir.AluOpType.add)
                nc.vector.tensor_scalar(out=drop[:], in0=rkQ[:], scalar1=float(CAPQ),
                                        scalar2=BIG, op0=mybir.AluOpType.is_ge,
                                        op1=mybir.AluOpType.mult)
                nc.vector.tensor_tensor(out=rkQ[:], in0=rkQ[:], in1=drop[:],
                                        op=mybir.AluOpType.add)
                nc.vector.tensor_scalar(out=drop[:], in0=eidx[:], scalar1=float(CAPQ),
                                        scalar2=None, op0=mybir.AluOpType.mult)
                nc.vector.tensor_tensor(out=rkQ[:], in0=rkQ[:], in1=drop[:],
                                        op=mybir.AluOpType.add)
                # one-hot select across expert blocks
                pA = mt.tile([P, W], f32)
                pB = mt.tile([P, W], f32)
                nc.vector.tensor_tensor(out=pA[:], in0=rkQ[:], in1=m1b[:],
                                        op=mybir.AluOpType.mult)
                nc.vector.tensor_tensor(out=pB[:], in0=rkQ[:], in1=m2b[:],
                                        op=mybir.AluOpType.mult)
                for src in (pA, pB):
                    for e in range(1, E):
                        nc.vector.tensor_tensor(
                            out=src[:, 0:NL], in0=src[:, 0:NL],
                            in1=src[:, e * NL:(e + 1) * NL],
                            op=mybir.AluOpType.add)
                # payload rows: row-in-quarter + 1 = 256c + 128(j%2) + p + 1
                mye256 = mt.tile([P, 1], f32)
                nc.vector.tensor_scalar(out=mye256[:], in0=mye_sb[:],
                                        scalar1=float(QT), scalar2=1.0,
                                        op0=mybir.AluOpType.mult,
                                        op1=mybir.AluOpType.add)
                tokp1 = mt.tile([P, NL], f32)
                nc.vector.tensor_copy(
                    tokp1[:].rearrange("p (q i) -> p q i", q=NQ),
                    tokf[:].rearrange("p (q i) -> p q i", q=1)
                    .to_broadcast([P, NQ, 2]))
                nc.vector.tensor_scalar(out=tokp1[:], in0=tokp1[:],
                                        scalar1=mye256[:, 0:1], scalar2=None,
                                        op0=mybir.AluOpType.add)
                nc.vector.tensor_copy(oA[:], pA[:, 0:NL])
                nc.vector.tensor_copy(oB[:], pB[:, 0:NL])
                nc.vector.tensor_copy(payA[:, 0:2 * NL:2], tokp1[:])
                nc.vector.tensor_copy(payA[:, 1:2 * NL:2], meta_sb[:, 2 * E:3 * E])
                nc.vector.tensor_copy(payB[:, 0:2 * NL:2], tokp1[:])
                nc.vector.tensor_copy(payB[:, 1:2 * NL:2], meta_sb[:, 3 * E:4 * E])
                # quarter 0's staging + 4 scatters + RS-t0 ONLY: with no other
                # DGE calls in flight, the trigger waits on exactly these 4
                # completions (no semaphore-ring aliasing) and fires ~30us
                # earlier; quarters 1-3 are chained into the FFN section.
                scatter_quarter(0, None)

            # ---------- per-quarter decode tiles ----------
            slot_toks = [cpool.tile([P, NSQ], i32, name=f"stok{q}")
                         for q in range(NQ)]
            slot_ws = [cpool.tile([P, NSQ], f32, name=f"sw{q}")
                       for q in range(NQ)]

            # =========== expert FFN (bf16, fp32 accumulate) ===========
            with tc.tile_pool(name="dq", bufs=1) as dq, \
                 tc.tile_pool(name="ffn", bufs=2) as ffn, \
                 tc.tile_pool(name="ffg", bufs=2) as ffg, \
                 tc.tile_pool(name="ffp", bufs=4, space="PSUM") as ffp:
                xgTs = [ffn.tile([P, DC, GRP], bf16, name=f"xgT_{g}", bufs=1)
                        for g in range(NGRP)]

                def gather_quarter(q):
                    # decode my slot table for quarter q (smf load on scalar:
                    # it waits on RS-t_q and must not block the gather queue)
                    smf = dq.tile([P, NSQ, 2], f32, tag="smf", bufs=2)
                    nc.scalar.dma_start(out=smf[:], in_=sms[q].ap().rearrange(
                        "(s p) w -> p s w", p=P))
                    nc.vector.tensor_copy(slot_ws[q][:], smf[:, :, 1])
                    tdec = dq.tile([P, NSQ], f32, tag="tdec", bufs=2)
                    empt = dq.tile([P, NSQ], f32, tag="empt", bufs=2)
                    nc.vector.tensor_scalar(out=empt[:], in0=smf[:, :, 0],
                                            scalar1=0.0,
                                            scalar2=float(PAD_TOK + 1),
                                            op0=mybir.AluOpType.is_equal,
                                            op1=mybir.AluOpType.mult)
                    nc.vector.tensor_scalar(out=tdec[:], in0=smf[:, :, 0],
                                            scalar1=-1.0, scalar2=None,
                                            op0=mybir.AluOpType.add)
                    nc.vector.tensor_tensor(out=tdec[:], in0=tdec[:], in1=empt[:],
                                            op=mybir.AluOpType.add)
                    nc.vector.tensor_copy(slot_toks[q][:], tdec[:])
                    for s in range(NSQ):
                        gt = NSQ * q + s
                        xg = ffg.tile([P, D], bf16, tag="xg", bufs=4)
                        nc.gpsimd.indirect_dma_start(
                            out=xg[:], out_offset=None,
                            in_=xqs[q].ap(),
                            in_offset=bass.IndirectOffsetOnAxis(
                                ap=slot_toks[q][:, s:s + 1], axis=0),
                            bounds_check=RQ - 1,
                            oob_is_err=False,
                        )
                        # xbar: xgT[p, dc, st*128+t] = xg[t, dc*128+p]
                        nc.sync.dma_start_transpose(
                            out=xgTs[gt // 4][:, :, (gt % 4) * P:(gt % 4 + 1) * P],
                            in_=xg[:])
                    return xg

                seed = gather_quarter(0)
                for q in range(1, NQ):
                    scatter_quarter(q, seed)
                    seed = gather_quarter(q)

                for g in range(NGRP):
                    xgT = xgTs[g]
                    # mm1 + gelu -> hT
                    hT = ffn.tile([P, FC, GRP], bf16, tag="hT")
                    for fci in range(FC):
                        hp = ffp.tile([P, GRP], f32, space="PSUM", tag="hp")
                        for dci in range(DC):
                            nc.tensor.matmul(out=hp[:],
                                             lhsT=w1_sb[:, dci, fci, :],
                                             rhs=xgT[:, dci, :],
                                             start=(dci == 0), stop=(dci == DC - 1))
                        nc.scalar.activation(out=hT[:, fci, :], in_=hp[:],
                                             func=mybir.ActivationFunctionType.Gelu,
                                             bias=b1_sb[:, fci:fci + 1], scale=1.0)
                    # mm2 -> (+bias)*gate -> oT; xbar-transpose immediately
                    oT = ffn.tile([P, DC, GRP], bf16, tag="oT")
                    owg = ffn.tile([P, GRP // P, D], bf16, tag="owg")
                    for dci in range(DC):
                        op = ffp.tile([P, GRP], f32, space="PSUM", tag="op")
                        for fci in range(FC):
                            nc.tensor.matmul(out=op[:],
                                             lhsT=w2_sb[:, fci, dci, :],
                                             rhs=hT[:, fci, :],
                                             start=(fci == 0), stop=(fci == FC - 1))
                        nc.vector.tensor_scalar(out=oT[:, dci, :], in0=op[:],
                                                scalar1=b2_sb[:, dci:dci + 1],
                                                scalar2=None,
                                                op0=mybir.AluOpType.add)
                        # owg[p, st, dc*128+jj] = oT[jj, dc, st*128+p]
                        nc.sync.dma_start_transpose(
                            out=owg[:, :, dci * P:(dci + 1) * P],
                            in_=oT[:, dci, :])
                    for st in range(GRP // P):
                        gt = g * (GRP // P) + st
                        q, s = gt // NSQ, gt % NSQ
                        ow = ffg.tile([P, D], bf16, tag="ow", bufs=3)
                        nc.vector.tensor_scalar(out=ow[:], in0=owg[:, st, :],
                                                scalar1=slot_ws[q][:, s:s + 1],
                                                scalar2=None,
                                                op0=mybir.AluOpType.mult)
                        nc.gpsimd.indirect_dma_start(
                            out=rss[q].ap(),
                            out_offset=bass.IndirectOffsetOnAxis(
                                ap=slot_toks[q][:, s:s + 1], axis=0),
                            in_=ow[:],
                            in_offset=None,
                            bounds_check=RQ - 1,
                            oob_is_err=False,
                        )
                        if s == NSQ - 1:
                            # quarter q fully scattered -> combine RS
                            nc.gpsimd.collective_compute(
                                "ReduceScatter", mybir.AluOpType.add,
                                replica_groups=[list(range(N_CORES))],
                                ins=[rss[q].ap().opt()], outs=[rsos[q].ap().opt()],
                            )
                # copyouts LAST, fenced on the final group's output so the
                # scheduler cannot hoist their loads ahead of FFN gelu work
                # on the scalar queue (they run under RS-rs3's window)
                fence = ffg.tile([1, 1], f32, tag="fence", bufs=1)
                nc.vector.tensor_copy(fence[:], ow[0:1, 0:1])
                for q in range(NQ):
                    for h in range(QT // P):
                        eng = nc.scalar if h % 2 == 0 else nc.sync
                        cb = ffg.tile([P, D], bf16, tag="cb", bufs=2)
                        nc.vector.tensor_copy(cb[0:1, 0:1], fence[:])
                        eng.dma_start(
                            out=cb[:], in_=rsos[q].ap()[h * P:(h + 1) * P, :])
                        ob = ffg.tile([P, D], f32, tag="ob", bufs=2)
                        nc.vector.tensor_copy(ob[:], cb[:])
                        eng.dma_start(
                            out=out_shard.ap()
                            [q * QT + h * P:q * QT + (h + 1) * P, :],
                            in_=ob[:])

    nc.finalize()
    return nc


_NC_CACHE = None
TRACE = False
LAST_EXEC_NS = None
LAST_TRACE_DIR = None


def kernel(x, router_w, router_b, w1, b1, w2, b2):
    global _NC_CACHE
    x = np.ascontiguousarray(np.asarray(x, np.float32))
    router_w = np.ascontiguousarray(np.asarray(router_w, np.float32))
    router_b = np.asarray(router_b, np.float32)
    w1 = np.asarray(w1, np.float32)
    b1 = np.asarray(b1, np.float32)
    w2 = np.asarray(w2, np.float32)
    b2 = np.asarray(b2, np.float32)

    xf = x.reshape(T, D)
    xbf = xf.astype(ml_dtypes.bfloat16)
    # quarter-permuted replicas: xq[q][c*QT + j] = x[c*TPC + q*QT + j]
    xq4 = xbf.reshape(N_CORES, NQ, QT, D)
    xq_arrs = [np.ascontiguousarray(xq4[:, q].reshape(RQ, D)) for q in range(NQ)]

    in_maps = []
    for c in range(N_CORES):
        xs = xf[c * TPC:(c + 1) * TPC]                      # [TPC, D]
        # two contiguous halves: [P, (h c t)] with t in [0, TPC/2)
        xT = np.ascontiguousarray(
            xs.T.reshape(DC, P, 2, TPC // 2).transpose(1, 2, 0, 3)
        ).reshape(P, DC * TPC)
        w1t = np.ascontiguousarray(
            w1[c].reshape(DC, P, FC, P).transpose(1, 0, 2, 3)
        ).astype(ml_dtypes.bfloat16).reshape(P, DC * FC * P)
        w2t = np.ascontiguousarray(
            w2[c].reshape(FC, P, DC, P).transpose(1, 0, 2, 3)
        ).astype(ml_dtypes.bfloat16).reshape(P, FC * DC * P)
        myr = np.zeros((E, 1), np.float32)
        myr[c, 0] = 1.0
        im = {
            "xT_p": xT,
            "rw": router_w,
            "rb_c": np.ascontiguousarray(router_b.reshape(E, 1)),
            "my_e": np.full((P, 1), float(c), np.float32),
            "myrow": myr,
            "w1_p": w1t,
            "b1_p": np.ascontiguousarray(b1[c].reshape(FC, P).T),
            "w2_p": w2t,
            "b2_p": np.ascontiguousarray(b2[c].reshape(DC, P).T),
        }
        for q in range(NQ):
            im[f"xq{q}"] = xq_arrs[q]
        in_maps.append(im)

    global LAST_EXEC_NS, LAST_TRACE_DIR
    if _NC_CACHE is None:
        _NC_CACHE = build_kernel()
    import tempfile
    td = tempfile.mkdtemp(prefix="moe_trace_") if TRACE else None
    res = run_bass_kernel_spmd(_NC_CACHE, in_maps, list(range(N_CORES)),
                               trace=TRACE, tmpdir=td)
    LAST_EXEC_NS = getattr(res, "exec_time_ns", None)
    LAST_TRACE_DIR = td
    out = np.concatenate([res.results[c]["out_shard"] for c in range(N_CORES)], axis=0)
    return out.reshape(B, S, D)
